# revision 1
# baseline (speedup 1.0000x reference)
"""CLIP attention block (LN(attn(x) @ W_out)) on 8 TRN2 NeuronCores.

Problem (hardcoded): x [4, 2048, 1024] f32, mask [4, 2048] bool,
w_qkv [1024, 3072], w_out [1024, 1024], ln_g [1024].
16 heads x 64 dim, causal, scale = 1/8. Output [4, 2048, 1024] f32.

Sharding: core = (batch b, parity rho). Each core computes the final
output rows for queries g of batch b with g % 2 == rho (1024 tokens).
Interleaving queries by parity makes the causal workload identical on
every core (SPMD-uniform static program): local query block i (128 rows,
globals 256*i + 2*r + rho) attends keys < 256*(i+1), so block i needs
exactly 2*(i+1) key-chunks of 128 regardless of rho; only the diagonal
mask tile differs per core, and that is input data.

Per-core plan (all matmuls in float32r: 1 cyc/row at N>=256, ~1.5e-4 rel):
  QT[inner, 1024] = (Wq^T chunks) @ xqT        (Q pre-scaled by 1/8)
  KT[inner, 2048], V[tok, inner] from xfT; V stored augmented per head
  as [V_h | m] (m = key padding multiplier) so the AV matmul's 65th
  output row accumulates the softmax denominator for free.
  S^T[k, q] = KT-slice^T @ QT-slice  (k on partitions -> exp needs no
  transposes anywhere; softmax uses no max-subtraction: |S| <~ 10).
  P^T = exp(S^T + causal mask), O^T[65, q] = sum_kc Vaug^T @ P^T.
  O^T rows 0..63 scaled by 1/denom (row 64): reciprocal + gpsimd
  partition_broadcast + DVE multiply, into OT[inner, 1024].
  Z[tok, dim] = sum_hp OT-slice^T @ w_out chunk; layernorm * gamma; out.
K^T/Q^T/O^T stream through DRAM scratch so SBUF holds only V resident;
DMAs are consolidated into ~100 large transfers (HWDGE issue is ~650ns
per dma_start and was the v1 bottleneck).
"""

import threading

import numpy as np

import concourse.bass as bass
import concourse.mybir as mybir
import concourse.tile as tile
from concourse import bacc
import concourse.bass_utils as bass_utils

F32 = mybir.dt.float32
F32R = mybir.dt.float32r

B, N, DIM = 4, 2048, 1024
HEADS, DH = 16, 64
INNER = HEADS * DH          # 1024
SCALE = DH ** -0.5          # 0.125
LOC = N // 2                # 1024 local query tokens per core
NEG = -1.0e30
EPS = 1e-5

NC = 8                      # cores
HP = HEADS // 2             # 8 head pairs
KC = N // 128               # 16 key chunks
G = LOC // 256              # 4 q-groups of 256


def build(reps=1):
    nc = bacc.Bacc("TRN2", target_bir_lowering=False, debug=False, num_devices=NC)

    xfT = nc.dram_tensor("xfT", [DIM, N], F32R, kind="ExternalInput").ap()
    xqT = nc.dram_tensor("xqT", [DIM, LOC], F32R, kind="ExternalInput").ap()
    wq = nc.dram_tensor("wq", [DIM, INNER], F32R, kind="ExternalInput").ap()
    wk = nc.dram_tensor("wk", [DIM, INNER], F32R, kind="ExternalInput").ap()
    wv = nc.dram_tensor("wv", [DIM, INNER], F32R, kind="ExternalInput").ap()
    wout = nc.dram_tensor("wout", [INNER, DIM], F32R, kind="ExternalInput").ap()
    lng = nc.dram_tensor("lng", [1, DIM], F32, kind="ExternalInput").ap()
    dmask = nc.dram_tensor("dmask", [128, 256], F32, kind="ExternalInput").ap()
    mvecT = nc.dram_tensor("mvecT", [128, KC], F32, kind="ExternalInput").ap()
    out = nc.dram_tensor("out", [LOC, DIM], F32, kind="ExternalOutput").ap()

    with nc.allow_low_precision(reason="float32r matmul staging"), \
         tile.TileContext(nc) as tc:
        for _ in range(reps):
            _build_body(nc, tc, xfT, xqT, wq, wk, wv, wout, lng, dmask, mvecT, out)

    nc.compile()
    return nc


def _build_body(nc, tc, xfT, xqT, wq, wk, wv, wout, lng, dmask, mvecT, out):
    mm = nc.tensor.matmul
    A = mybir.ActivationFunctionType

    # ---------------- prep: small residents ----------------
    res = tc.alloc_tile_pool(name="res", bufs=1)
    DM = res.tile([128, 256], F32, tag="DM")
    nc.sync.dma_start(DM[:], dmask[:])
    MV = res.tile([128, KC], F32, tag="MV")
    nc.sync.dma_start(MV[:], mvecT[:])
    ones16 = res.tile([128, HEADS, 1], F32, tag="ones16")
    nc.vector.memset(ones16[:], 1.0)
    epst = res.tile([128, 1], F32, tag="epst")
    nc.vector.memset(epst[:], EPS)

    # V: 16 token-chunk tiles, per head [64 V cols | padding-multiplier col]
    vpool = tc.alloc_tile_pool(name="vres", bufs=1)
    V = [vpool.tile([128, HEADS, DH + 1], F32R, tag=f"V{i}", name=f"V{i}")
         for i in range(KC)]

    # DRAM scratch for K^T, Q^T, O^T (streamed back per head-pair)
    dram = tc.alloc_tile_pool(name="dram", bufs=1, space="DRAM")
    KTd = [dram.tile([128, N], F32R, tag=f"ktd{hp}", name=f"ktd{hp}")
           for hp in range(HP)]
    QTd = [dram.tile([128, LOC], F32R, tag=f"qtd{hp}", name=f"qtd{hp}")
           for hp in range(HP)]

    pps = tc.alloc_tile_pool(name="pps", bufs=2, space="PSUM")

    # ---------------- phase V: V projection ----------------
    xf_pool = tc.alloc_tile_pool(name="xf", bufs=1)
    XF = [xf_pool.tile([128, N], F32R, tag=f"xf{dc}", name=f"xf{dc}")
          for dc in range(8)]
    wv_pool = tc.alloc_tile_pool(name="wv", bufs=1)
    WV = [wv_pool.tile([128, INNER], F32R, tag=f"wv{dc}", name=f"wv{dc}")
          for dc in range(8)]
    for dc in range(8):  # interleave so the first psum chain starts early
        eng = nc.sync if dc % 2 == 0 else nc.gpsimd
        eng.dma_start(XF[dc][:], xfT[dc * 128:(dc + 1) * 128, :])
        eng2 = nc.gpsimd if dc % 2 == 0 else nc.sync
        eng2.dma_start(WV[dc][:], wv[dc * 128:(dc + 1) * 128, :])

    for tci in range(KC):
        for ig in range(2):
            vp = pps.tile([128, 512], F32, tag="pp", name="vp")
            for dc in range(8):
                mm(vp[:], XF[dc][:, tci * 128:(tci + 1) * 128],
                   WV[dc][:, ig * 512:(ig + 1) * 512],
                   start=(dc == 0), stop=(dc == 7))
            dst = V[tci][:, ig * 8:(ig + 1) * 8, 0:DH]
            nc.vector.tensor_scalar_mul(
                dst, vp[:].rearrange("p (h d) -> p h d", d=DH),
                MV[:, tci:tci + 1])
        nc.vector.tensor_scalar_mul(
            V[tci][:, :, DH:DH + 1], ones16[:], MV[:, tci:tci + 1])
    wv_pool.release()

    wq_r = wq.rearrange("(dc p) j -> p dc j", p=128)
    wk_r = wk.rearrange("(dc p) j -> p dc j", p=128)

    # ---------------- phase Q: Q^T projection (pre-scaled) -> DRAM --------
    xq_pool = tc.alloc_tile_pool(name="xq", bufs=1)
    XQ = [xq_pool.tile([128, LOC], F32R, tag=f"xq{dc}", name=f"xq{dc}")
          for dc in range(8)]
    wqc_pool = tc.alloc_tile_pool(name="wqc", bufs=2)
    for dc in range(8):
        nc.sync.dma_start(XQ[dc][:], xqT[dc * 128:(dc + 1) * 128, :])
    qst_pool = tc.alloc_tile_pool(name="qst", bufs=2)
    for hp in range(HP):
        wqc = wqc_pool.tile([128, 8, 128], F32R, tag="wqc", name="wqc")
        nc.gpsimd.dma_start(wqc[:], wq_r[:, :, hp * 128:(hp + 1) * 128])
        qst = qst_pool.tile([128, LOC], F32R, tag="qst", name="qst")
        for tg in range(2):
            qp = pps.tile([128, 512], F32, tag="pp", name="qp")
            for dc in range(8):
                mm(qp[:], wqc[:, dc, :],
                   XQ[dc][:, tg * 512:(tg + 1) * 512],
                   start=(dc == 0), stop=(dc == 7))
            nc.vector.tensor_scalar_mul(qst[:, tg * 512:(tg + 1) * 512],
                                        qp[:], SCALE)
        nc.gpsimd.dma_start(QTd[hp][:], qst[:])
    qst_pool.release()
    wqc_pool.release()
    xq_pool.release()

    # ---------------- attention pools (opened before K so the K tail can
    # overlap attention without SBUF-address reuse serialization) ----------
    pps.release()
    ots_pool = tc.alloc_tile_pool(name="ots", bufs=1)
    ktp_pool = tc.alloc_tile_pool(name="ktp", bufs=2)
    qtp_pool = tc.alloc_tile_pool(name="qtp", bufs=2)
    pt_pool = tc.alloc_tile_pool(name="pt", bufs=2)
    rc_pool = tc.alloc_tile_pool(name="rc", bufs=1)
    st_ps = tc.alloc_tile_pool(name="stps", bufs=3, space="PSUM")
    o_ps = tc.alloc_tile_pool(name="ops", bufs=2, space="PSUM")

    # ---------------- phase K: K^T projection -> DRAM (overlaps attention) -
    wkc_pool = tc.alloc_tile_pool(name="wkc", bufs=1)
    kst_pool = tc.alloc_tile_pool(name="kst", bufs=2)
    for hp in range(HP):
        wkc = wkc_pool.tile([128, 8, 128], F32R, tag="wkc", name="wkc")
        nc.gpsimd.dma_start(wkc[:], wk_r[:, :, hp * 128:(hp + 1) * 128])
        for tg in range(4):
            kp = st_ps.tile([128, 512], F32, tag="st", name="kp")
            for dc in range(8):
                mm(kp[:], wkc[:, dc, :],
                   XF[dc][:, tg * 512:(tg + 1) * 512],
                   start=(dc == 0), stop=(dc == 7))
            kst = kst_pool.tile([128, 512], F32R, tag="kst", name="kst")
            nc.vector.tensor_copy(kst[:], kp[:])
            nc.gpsimd.dma_start(KTd[hp][:, tg * 512:(tg + 1) * 512], kst[:])
    kst_pool.release()
    wkc_pool.release()

    # ---------------- attention ----------------
    OTS = []
    for hp in range(HP):
        KTt = ktp_pool.tile([128, N], F32R, tag="ktt", name="KTt")
        nc.sync.dma_start(KTt[:], KTd[hp][:])
        QTt = qtp_pool.tile([128, LOC], F32R, tag="qtt", name="QTt")
        nc.sync.dma_start(QTt[:], QTd[hp][:])
        OTt = ots_pool.tile([128, LOC], F32R, tag=f"ott{hp}", name=f"OTt{hp}")
        OTS.append(OTt)
        for h2 in range(2):
            h = 2 * hp + h2
            hs = slice(h2 * DH, (h2 + 1) * DH)
            for gg in range(2):  # q-groups of 512
                n_kc = 8 * (gg + 1)
                nstrip = 4 * (gg + 1)
                op = o_ps.tile([DH + 1, 512], F32, tag="o", name="op")
                sts = {}

                def emit_scores(t):
                    st = st_ps.tile([128, 1024], F32, tag="st", name="st")
                    sts[t] = st
                    for d in range(2):
                        kc = 2 * t + d
                        mm(st[:, d * 512:(d + 1) * 512],
                           KTt[hs, kc * 128:(kc + 1) * 128],
                           QTt[hs, gg * 512:(gg + 1) * 512],
                           start=True, stop=True)

                for t in range(nstrip):
                    if t == 0:
                        for w in range(min(3, nstrip)):
                            emit_scores(w)
                    elif t + 2 < nstrip:
                        emit_scores(t + 2)
                    st = sts.pop(t)
                    pt = pt_pool.tile([128, 2, 512], F32R, tag="pt", name="pt")
                    td = t - 4 * gg  # diagonal position, >= 0 on masked strips
                    stv = st[:].rearrange("p (d q) -> p d q", d=2)
                    if td >= 0:
                        nc.vector.tensor_add(
                            stv[:, :, td * 128:(td + 1) * 128],
                            stv[:, :, td * 128:(td + 1) * 128],
                            DM[:].rearrange("p (d q) -> p d q", d=2))
                        if td > 0:  # zero fully-causally-invalid columns
                            nc.vector.memset(pt[:, :, 0:td * 128].bitcast(F32), 0.0)
                        nc.scalar.activation(pt[:, :, td * 128:512],
                                             stv[:, :, td * 128:512], A.Exp)
                    else:
                        nc.scalar.activation(pt[:, :, :], stv[:, :, :], A.Exp)
                    for d in range(2):
                        kc = 2 * t + d
                        mm(op[:], V[kc][:, h, :], pt[:, d, :],
                           start=(kc == 0), stop=(kc == n_kc - 1))
                rcp = rc_pool.tile([1, 512], F32, tag="rcp", name="rcp")
                nc.vector.reciprocal(rcp[:], op[DH:DH + 1, :])
                rbs = rc_pool.tile([DH, 512], F32, tag="rbs", name="rbs")
                nc.gpsimd.partition_broadcast(rbs[:], rcp[:])
                nc.vector.tensor_mul(
                    OTt[hs, gg * 512:(gg + 1) * 512], op[0:DH, :], rbs[:])

    rc_pool.release()
    pt_pool.release()
    qtp_pool.release()
    ktp_pool.release()
    o_ps.release()
    st_ps.release()

    # ---------------- out projection + layernorm ----------------
    gz_pool = tc.alloc_tile_pool(name="gz", bufs=1)
    grow = gz_pool.tile([1, DIM], F32, tag="grow")
    nc.sync.dma_start(grow[:], lng[:])
    GB = gz_pool.tile([128, DIM], F32, tag="GB")
    nc.gpsimd.partition_broadcast(GB[:], grow[:])
    wo_pool = tc.alloc_tile_pool(name="wo", bufs=2)
    stat_pool = tc.alloc_tile_pool(name="stat", bufs=2)
    stage_pool = tc.alloc_tile_pool(name="stage", bufs=2)
    z_ps = tc.alloc_tile_pool(name="zps", bufs=1, space="PSUM")

    for tbg in range(2):
        zps = {}
        for ti in range(4):
            for half in range(2):
                zps[(ti, half)] = z_ps.tile([128, 512], F32, tag=f"z{ti}{half}",
                                            name=f"z{ti}{half}")
        for hp in range(HP):
            wo = wo_pool.tile([128, DIM], F32R, tag="wo", name="wo")
            nc.sync.dma_start(wo[:], wout[hp * 128:(hp + 1) * 128, :])
            for ti in range(4):
                tb = tbg * 4 + ti
                for half in range(2):
                    mm(zps[(ti, half)][:],
                       OTS[hp][:, tb * 128:(tb + 1) * 128],
                       wo[:, half * 512:(half + 1) * 512],
                       start=(hp == 0), stop=(hp == HP - 1))
        for ti in range(4):
            tb = tbg * 4 + ti
            s_ = [stat_pool.tile([128, 1], F32, tag=f"s{half}", name=f"s{half}")
                  for half in range(2)]
            q_ = [stat_pool.tile([128, 1], F32, tag=f"q{half}", name=f"q{half}")
                  for half in range(2)]
            scr = stage_pool.tile([128, 512], F32, tag="scr", name="scr")
            for half in range(2):
                nc.vector.reduce_sum(s_[half][:], zps[(ti, half)][:],
                                     axis=mybir.AxisListType.X)
                nc.scalar.activation(scr[:], zps[(ti, half)][:], A.Square,
                                     accum_out=q_[half][:])
            mean = stat_pool.tile([128, 1], F32, tag="mean", name="mean")
            nc.vector.tensor_add(mean[:], s_[0][:], s_[1][:])
            nc.vector.tensor_scalar_mul(mean[:], mean[:], 1.0 / DIM)
            msq = stat_pool.tile([128, 1], F32, tag="msq", name="msq")
            nc.vector.tensor_add(msq[:], q_[0][:], q_[1][:])
            nc.vector.tensor_scalar_mul(msq[:], msq[:], 1.0 / DIM)
            var = stat_pool.tile([128, 1], F32, tag="var", name="var")
            nc.vector.tensor_mul(var[:], mean[:], mean[:])
            nc.vector.tensor_sub(var[:], msq[:], var[:])
            std = stat_pool.tile([128, 1], F32, tag="std", name="std")
            nc.scalar.activation(std[:], var[:], A.Sqrt, bias=epst[:])
            rstd = stat_pool.tile([128, 1], F32, tag="rstd", name="rstd")
            nc.vector.reciprocal(rstd[:], std[:])
            nmr = stat_pool.tile([128, 1], F32, tag="nmr", name="nmr")
            nc.vector.tensor_mul(nmr[:], mean[:], rstd[:])
            nc.vector.tensor_scalar_mul(nmr[:], nmr[:], -1.0)
            outb = stage_pool.tile([128, DIM], F32, tag="outb", name="outb")
            for half in range(2):
                zn = stage_pool.tile([128, 512], F32, tag=f"zn{half}",
                                     name=f"zn{half}")
                nc.scalar.activation(zn[:], zps[(ti, half)][:], A.Identity,
                                     bias=nmr[:], scale=rstd[:])
                nc.vector.tensor_mul(outb[:, half * 512:(half + 1) * 512],
                                     zn[:], GB[:, half * 512:(half + 1) * 512])
            nc.gpsimd.dma_start(out[tb * 128:(tb + 1) * 128, :], outb[:])

    z_ps.release()
    stage_pool.release()
    stat_pool.release()
    wo_pool.release()
    gz_pool.release()
    ots_pool.release()
    xf_pool.release()
    dram.release()
    vpool.release()
    res.release()


def make_in_maps(x, mask, w_qkv, w_out, ln_g):
    x = np.asarray(x, dtype=np.float32)
    mask_np = np.asarray(mask)
    w_qkv = np.asarray(w_qkv, dtype=np.float32)
    w_out = np.ascontiguousarray(np.asarray(w_out, dtype=np.float32))
    ln_g = np.asarray(ln_g, dtype=np.float32)

    wq = np.ascontiguousarray(w_qkv[:, :INNER])
    wk = np.ascontiguousarray(w_qkv[:, INNER:2 * INNER])
    wv = np.ascontiguousarray(w_qkv[:, 2 * INNER:])
    lng = np.ascontiguousarray(ln_g[None, :])

    # diagonal mask pair [128 k, (2 kc-halves) x 128 q]: MT0 | MT1
    kk = np.arange(128)[:, None]
    r = np.arange(128)[None, :]
    dmasks = {}
    for rho in (0, 1):
        m0 = np.where(kk <= 2 * r + rho, 0.0, NEG).astype(np.float32)
        m1 = np.where(kk + 128 <= 2 * r + rho, 0.0, NEG).astype(np.float32)
        dmasks[rho] = np.ascontiguousarray(np.concatenate([m0, m1], axis=1))

    in_maps = []
    for b in range(B):
        xf = x[b]  # [N, DIM]
        xfT = np.ascontiguousarray(xf.T)
        mv = mask_np[b].astype(np.float32)  # [N]
        mvecT = np.ascontiguousarray(mv.reshape(KC, 128).T)  # [128, KC]
        for rho in (0, 1):
            xqT = np.ascontiguousarray(xf[rho::2, :].T)
            in_maps.append({
                "xfT": xfT, "xqT": xqT, "wq": wq, "wk": wk, "wv": wv,
                "wout": w_out, "lng": lng, "dmask": dmasks[rho],
                "mvecT": mvecT,
            })
    return in_maps


_CACHE = {}
_LOCK = threading.Lock()


def _get_nc():
    with _LOCK:
        if "nc" not in _CACHE:
            _CACHE["nc"] = build()
    return _CACHE["nc"]


def kernel(x, mask, w_qkv, w_out, ln_g):
    in_maps = make_in_maps(x, mask, w_qkv, w_out, ln_g)
    nc = _get_nc()
    res = bass_utils.run_bass_kernel_spmd(nc, in_maps, core_ids=list(range(NC)))

    final = np.empty((B, N, DIM), dtype=np.float32)
    for b in range(B):
        for rho in (0, 1):
            final[b, rho::2, :] = res.results[2 * b + rho]["out"]
    return final



# revision 9
# speedup vs baseline: 1.2730x; 1.2730x over previous
"""CLIP attention block (LN(attn(x) @ W_out)) on 8 TRN2 NeuronCores.

Problem (hardcoded): x [4, 2048, 1024] f32, mask [4, 2048] bool,
w_qkv [1024, 3072], w_out [1024, 1024], ln_g [1024].
16 heads x 64 dim, causal, scale = 1/8. Output [4, 2048, 1024] f32.

Sharding: core = (batch b, parity rho); core computes output rows for
queries of batch b with token % 2 == rho (1024 tokens).

Layout: tokens are PERMUTED on host to [even | odd] so the core's query
set is a contiguous 1024-column slice of xfT and the causal structure
versus permuted key index k' is triangular with unit slope in each of
the two key regions (even keys: k' <= q'; odd keys: k' < q' for rho=0,
k' <= q' for rho=1). Keys/V/KT all live in permuted order (attention is
permutation-invariant over keys).

Pipeline (single pass, engineered for PE density / HAM warmth):
  prologue: V = x @ Wv (bf16, per-head-aug denominator column), K0/Q0.
  loop over 8 head pairs: packed 2-head score matmuls (f32r, K=64 row
  tiles at partitions 0/64 run concurrently), exp on ACT (scale=1/8
  folded in), AV matmuls in bf16 against V chunks (aug row 65
  accumulates the softmax denominator), per-head normalize via DVE
  reciprocal + gpsimd partition_broadcast; K/Q projection for the NEXT
  head pair is emitted between attention groups so the tensor engine
  always has dense independent work (keeps the HAM clock-gate at 8/8).
  Causally-dead column ranges are trimmed from score/AV streams and exp.
  epilogue: out proj over head pairs into 2x8 PSUM banks + layernorm.
"""

import threading

import numpy as np

import concourse.bass as bass
import concourse.mybir as mybir
import concourse.tile as tile
from concourse import bacc
import concourse.bass_utils as bass_utils

F32 = mybir.dt.float32
F32R = mybir.dt.float32r
BF16 = mybir.dt.bfloat16

B, N, DIM = 4, 2048, 1024
HEADS, DH = 16, 64
INNER = HEADS * DH          # 1024
SCALE = DH ** -0.5          # 0.125
LOC = N // 2                # 1024 local query tokens per core
NEG = -1.0e30
EPS = 1e-5

NC = 8                      # cores
HP = HEADS // 2             # 8 head pairs
KC = N // 128               # 16 key chunks (permuted order: 0-7 even, 8-15 odd)


def build(reps=1):
    nc = bacc.Bacc("TRN2", target_bir_lowering=False, debug=False, num_devices=NC)

    xfT = nc.dram_tensor("xfT", [DIM, N], F32R, kind="ExternalInput").ap()
    wq = nc.dram_tensor("wq", [DIM, INNER], F32R, kind="ExternalInput").ap()
    wk = nc.dram_tensor("wk", [DIM, INNER], F32R, kind="ExternalInput").ap()
    wv = nc.dram_tensor("wv", [DIM, INNER], F32R, kind="ExternalInput").ap()
    wout = nc.dram_tensor("wout", [INNER, DIM], F32R, kind="ExternalInput").ap()
    lng = nc.dram_tensor("lng", [1, DIM], F32, kind="ExternalInput").ap()
    dmask = nc.dram_tensor("dmask", [128, 2, 2, 128], F32, kind="ExternalInput").ap()
    mvecT = nc.dram_tensor("mvecT", [128, KC], F32, kind="ExternalInput").ap()
    out = nc.dram_tensor("out", [LOC, DIM], F32, kind="ExternalOutput").ap()

    with nc.allow_low_precision(reason="bf16 attention staging"), \
         tile.TileContext(nc) as tc:
        for _ in range(reps):
            _build_body(nc, tc, xfT, wq, wk, wv, wout, lng, dmask, mvecT, out)

    nc.compile()
    return nc


def _strips(gg):
    """Strips for q-group gg (512 local queries): (region, kci, trim s).

    region 0 = even keys (KT cols kci*128), region 1 = odd keys (KT cols
    (8+kci)*128). kci < 4*(gg+1). Diagonal strips (kci >= 4*gg) only have
    valid q columns [128*(kci-4*gg), 512)."""
    out = []
    for kci in range(4 * (gg + 1)):
        rel = kci - 4 * gg
        s = 128 * rel if rel >= 0 else 0
        out.append((0, kci, s))
        out.append((1, kci, s))
    return out


def _build_body(nc, tc, xfT, wq, wk, wv, wout, lng, dmask, mvecT, out):
    mm = nc.tensor.matmul
    A = mybir.ActivationFunctionType

    # ---------------- small residents ----------------
    res = tc.alloc_tile_pool(name="res", bufs=1)
    DM = res.tile([128, 2, 2, 128], F32, tag="DM")   # [k, region, head, q]
    nc.sync.dma_start(DM[:], dmask[:])
    MV = res.tile([128, KC], F32, tag="MV")
    nc.sync.dma_start(MV[:], mvecT[:])
    ones16 = res.tile([128, HEADS, 1], F32, tag="ones16")
    nc.vector.memset(ones16[:], 1.0)
    epst = res.tile([128, 1], F32, tag="epst")
    nc.vector.memset(epst[:], EPS)

    # V resident: 16 key chunks, per head [64 V cols | denominator col]
    vpool = tc.alloc_tile_pool(name="vres", bufs=1)
    V = [vpool.tile([128, HEADS, DH + 1], BF16, tag=f"V{i}", name=f"V{i}")
         for i in range(KC)]

    # OTS outlives the attention-phase pools below (LIFO release order)
    ots_pool = tc.alloc_tile_pool(name="ots", bufs=1)

    pps = tc.alloc_tile_pool(name="pps", bufs=2, space="PSUM")

    # ---------------- phase V: V projection ----------------
    xf_pool = tc.alloc_tile_pool(name="xf", bufs=1)
    XF = [xf_pool.tile([128, N], F32R, tag=f"xf{dc}", name=f"xf{dc}")
          for dc in range(8)]
    wv_pool = tc.alloc_tile_pool(name="wv", bufs=1)
    WV = [wv_pool.tile([128, INNER], F32R, tag=f"wv{dc}", name=f"wv{dc}")
          for dc in range(8)]
    for dc in range(8):
        eng = nc.sync if dc % 2 == 0 else nc.gpsimd
        eng.dma_start(XF[dc][:], xfT[dc * 128:(dc + 1) * 128, :])
        eng2 = nc.gpsimd if dc % 2 == 0 else nc.sync
        eng2.dma_start(WV[dc][:], wv[dc * 128:(dc + 1) * 128, :])

    for tci in range(KC):
        for ig in range(2):
            vp = pps.tile([128, 512], F32, tag="pp", name="vp")
            for dc in range(8):
                mm(vp[:], XF[dc][:, tci * 128:(tci + 1) * 128],
                   WV[dc][:, ig * 512:(ig + 1) * 512],
                   start=(dc == 0), stop=(dc == 7))
            dst = V[tci][:, ig * 8:(ig + 1) * 8, 0:DH]
            nc.vector.tensor_scalar_mul(
                dst, vp[:].rearrange("p (h d) -> p h d", d=DH),
                MV[:, tci:tci + 1])
        nc.vector.tensor_scalar_mul(
            V[tci][:, :, DH:DH + 1], ones16[:], MV[:, tci:tci + 1])
    wv_pool.release()

    wq_r = wq.rearrange("(dc p) j -> p dc j", p=128)
    wk_r = wk.rearrange("(dc p) j -> p dc j", p=128)

    # ---------------- loop pools ----------------
    wkc_pool = tc.alloc_tile_pool(name="wkc", bufs=2)
    wqc_pool = tc.alloc_tile_pool(name="wqc", bufs=2)
    ktp_pool = tc.alloc_tile_pool(name="ktp", bufs=2)
    qtp_pool = tc.alloc_tile_pool(name="qtp", bufs=2)
    pt_pool = tc.alloc_tile_pool(name="pt", bufs=2)
    rc_pool = tc.alloc_tile_pool(name="rc", bufs=2)
    st_ps = tc.alloc_tile_pool(name="stps", bufs=2, space="PSUM")
    o_ps = tc.alloc_tile_pool(name="ops", bufs=1, space="PSUM")

    kts, qts = {}, {}

    def emit_kq(hp, part):
        """Emit K/Q projection work for head pair hp. part 0: wk dma +
        first 2 K chains; part 1: rest of K + Q."""
        if part == 0:
            wkc = wkc_pool.tile([128, 8, 128], F32R, tag="wkc", name="wkc")
            nc.gpsimd.dma_start(wkc[:], wk_r[:, :, hp * 128:(hp + 1) * 128])
            emit_kq.wkc = wkc
            wqc = wqc_pool.tile([128, 8, 128], F32R, tag="wqc", name="wqc")
            nc.sync.dma_start(wqc[:], wq_r[:, :, hp * 128:(hp + 1) * 128])
            emit_kq.wqc = wqc
            KTt = ktp_pool.tile([128, N], F32R, tag="ktt", name="KTt")
            kts[hp] = KTt
            tgs = range(0, 2)
        else:
            wkc = emit_kq.wkc
            KTt = kts[hp]
            tgs = range(2, 4)
        for tg in tgs:
            kp = pps.tile([128, 512], F32, tag="pp", name="kp")
            for dc in range(8):
                mm(kp[:], wkc[:, dc, :], XF[dc][:, tg * 512:(tg + 1) * 512],
                   start=(dc == 0), stop=(dc == 7))
            nc.vector.tensor_copy(KTt[:, tg * 512:(tg + 1) * 512], kp[:])
        if part == 1:
            wqc = emit_kq.wqc
            QTt = qtp_pool.tile([128, LOC], F32R, tag="qtt", name="QTt")
            qts[hp] = QTt
            for tg in range(2):
                qp = pps.tile([128, 512], F32, tag="pp", name="qp")
                for dc in range(8):
                    # queries = first half of the permuted tokens
                    mm(qp[:], wqc[:, dc, :],
                       XF[dc][:, tg * 512:(tg + 1) * 512],
                       start=(dc == 0), stop=(dc == 7))
                nc.vector.tensor_copy(QTt[:, tg * 512:(tg + 1) * 512], qp[:])

    emit_kq(0, 0)
    emit_kq(0, 1)

    # ---------------- attention loop ----------------
    OTS = []
    for hp in range(HP):
        KTt, QTt = kts.pop(hp), qts.pop(hp)
        OTt = ots_pool.tile([128, LOC], F32R, tag=f"ott{hp}", name=f"OTt{hp}")
        OTS.append(OTt)
        for gg in range(2):
            strips = _strips(gg)
            nstrip = len(strips)
            ops = [o_ps.tile([DH + 1, 512], F32, tag=f"o{h}", name=f"op{h}")
                   for h in range(2)]
            sts = {}

            def emit_scores(t):
                region, kci, s = strips[t]
                ck = kci if region == 0 else 8 + kci
                st2 = st_ps.tile([128, 2, 512], F32, tag="st", name="st")
                sts[t] = st2
                qsl = slice(gg * 512 + s, (gg + 1) * 512)
                for h in range(2):
                    hs = slice(h * DH, (h + 1) * DH)
                    mm(st2[:, h, s:512],
                       KTt[hs, ck * 128:(ck + 1) * 128], QTt[hs, qsl],
                       start=True, stop=True)
                return st2

            for t in range(nstrip):
                if t == 0:
                    emit_scores(0)
                    if nstrip > 1:
                        emit_scores(1)
                elif t + 1 < nstrip:
                    emit_scores(t + 1)
                region, kci, s = strips[t]
                ck = kci if region == 0 else 8 + kci
                st2 = sts.pop(t)
                rel = kci - 4 * gg
                if rel >= 0:  # diagonal strip: add mask band
                    nc.vector.tensor_add(
                        st2[:, :, s:s + 128], st2[:, :, s:s + 128],
                        DM[:, region])
                pt = pt_pool.tile([128, 2, 512], BF16, tag="pt", name="pt")
                nc.scalar.activation(pt[:, :, s:512], st2[:, :, s:512],
                                     A.Exp, scale=SCALE)
                for h in range(2):
                    mm(ops[h][:, s:512], V[ck][:, 2 * hp + h, :],
                       pt[:, h, s:512],
                       start=(t == 0), stop=(t == nstrip - 1))
            # interleave next head pair's projections with attention
            if hp + 1 < HP:
                emit_kq(hp + 1, gg)
            for h in range(2):
                rcp = rc_pool.tile([1, 512], F32, tag="rcp", name="rcp")
                nc.vector.reciprocal(rcp[:], ops[h][DH:DH + 1, :])
                rbs = rc_pool.tile([DH, 512], F32, tag="rbs", name="rbs")
                nc.gpsimd.partition_broadcast(rbs[:], rcp[:])
                nc.vector.tensor_mul(
                    OTt[h * DH:(h + 1) * DH, gg * 512:(gg + 1) * 512],
                    ops[h][0:DH, :], rbs[:])

    o_ps.release()
    st_ps.release()
    rc_pool.release()
    pt_pool.release()
    qtp_pool.release()
    ktp_pool.release()
    wqc_pool.release()
    wkc_pool.release()
    xf_pool.release()
    pps.release()

    # ---------------- out projection + layernorm ----------------
    gz_pool = tc.alloc_tile_pool(name="gz", bufs=1)
    grow = gz_pool.tile([1, DIM], F32, tag="grow")
    nc.sync.dma_start(grow[:], lng[:])
    GB = gz_pool.tile([128, DIM], F32, tag="GB")
    nc.gpsimd.partition_broadcast(GB[:], grow[:])
    wo_pool = tc.alloc_tile_pool(name="wo", bufs=2)
    stat_pool = tc.alloc_tile_pool(name="stat", bufs=2)
    stage_pool = tc.alloc_tile_pool(name="stage", bufs=2)
    z_ps = tc.alloc_tile_pool(name="zps", bufs=1, space="PSUM")

    for tbg in range(2):
        zps = {}
        for ti in range(4):
            for half in range(2):
                zps[(ti, half)] = z_ps.tile([128, 512], F32, tag=f"z{ti}{half}",
                                            name=f"z{ti}{half}")
        for hp in range(HP):
            wo = wo_pool.tile([128, DIM], F32R, tag="wo", name="wo")
            nc.sync.dma_start(wo[:], wout[hp * 128:(hp + 1) * 128, :])
            for ti in range(4):
                tb = tbg * 4 + ti
                for half in range(2):
                    mm(zps[(ti, half)][:],
                       OTS[hp][:, tb * 128:(tb + 1) * 128],
                       wo[:, half * 512:(half + 1) * 512],
                       start=(hp == 0), stop=(hp == HP - 1))
        for ti in range(4):
            tb = tbg * 4 + ti
            s_ = [stat_pool.tile([128, 1], F32, tag=f"s{half}", name=f"s{half}")
                  for half in range(2)]
            q_ = [stat_pool.tile([128, 1], F32, tag=f"q{half}", name=f"q{half}")
                  for half in range(2)]
            scr = stage_pool.tile([128, 512], F32, tag="scr", name="scr")
            for half in range(2):
                nc.vector.reduce_sum(s_[half][:], zps[(ti, half)][:],
                                     axis=mybir.AxisListType.X)
                nc.scalar.activation(scr[:], zps[(ti, half)][:], A.Square,
                                     accum_out=q_[half][:])
            mean = stat_pool.tile([128, 1], F32, tag="mean", name="mean")
            nc.vector.tensor_add(mean[:], s_[0][:], s_[1][:])
            nc.vector.tensor_scalar_mul(mean[:], mean[:], 1.0 / DIM)
            msq = stat_pool.tile([128, 1], F32, tag="msq", name="msq")
            nc.vector.tensor_add(msq[:], q_[0][:], q_[1][:])
            nc.vector.tensor_scalar_mul(msq[:], msq[:], 1.0 / DIM)
            var = stat_pool.tile([128, 1], F32, tag="var", name="var")
            nc.vector.tensor_mul(var[:], mean[:], mean[:])
            nc.vector.tensor_sub(var[:], msq[:], var[:])
            std = stat_pool.tile([128, 1], F32, tag="std", name="std")
            nc.scalar.activation(std[:], var[:], A.Sqrt, bias=epst[:])
            rstd = stat_pool.tile([128, 1], F32, tag="rstd", name="rstd")
            nc.vector.reciprocal(rstd[:], std[:])
            nmr = stat_pool.tile([128, 1], F32, tag="nmr", name="nmr")
            nc.vector.tensor_mul(nmr[:], mean[:], rstd[:])
            nc.vector.tensor_scalar_mul(nmr[:], nmr[:], -1.0)
            outb = stage_pool.tile([128, DIM], F32, tag="outb", name="outb")
            for half in range(2):
                zn = stage_pool.tile([128, 512], F32, tag=f"zn{half}",
                                     name=f"zn{half}")
                nc.scalar.activation(zn[:], zps[(ti, half)][:], A.Identity,
                                     bias=nmr[:], scale=rstd[:])
                nc.vector.tensor_mul(outb[:, half * 512:(half + 1) * 512],
                                     zn[:], GB[:, half * 512:(half + 1) * 512])
            nc.gpsimd.dma_start(out[tb * 128:(tb + 1) * 128, :], outb[:])

    z_ps.release()
    stage_pool.release()
    stat_pool.release()
    wo_pool.release()
    gz_pool.release()
    ots_pool.release()
    vpool.release()
    res.release()


def make_in_maps(x, mask, w_qkv, w_out, ln_g):
    x = np.asarray(x, dtype=np.float32)
    mask_np = np.asarray(mask)
    w_qkv = np.asarray(w_qkv, dtype=np.float32)
    w_out = np.ascontiguousarray(np.asarray(w_out, dtype=np.float32))
    ln_g = np.asarray(ln_g, dtype=np.float32)

    wq = np.ascontiguousarray(w_qkv[:, :INNER])
    wk = np.ascontiguousarray(w_qkv[:, INNER:2 * INNER])
    wv = np.ascontiguousarray(w_qkv[:, 2 * INNER:])
    lng = np.ascontiguousarray(ln_g[None, :])

    # Diagonal masks in permuted space. Core's queries are parity rho and
    # are placed FIRST in the permutation [rho tokens | 1-rho tokens], so
    # the query slice is always columns 0..1023. Region 0 = same-parity
    # keys (global 2k'+rho vs 2q'+rho: k' <= q'), region 1 = other-parity
    # keys (global 2k'+(1-rho) vs 2q'+rho: valid iff 2k'+1-rho <= 2q'+rho,
    # i.e. k' < q' + rho: rho=0 -> k' < q'; rho=1 -> k' <= q').
    kk = np.arange(128)[:, None]
    qq = np.arange(128)[None, :]
    m_inc = np.where(kk <= qq, 0.0, NEG).astype(np.float32)
    m_exc = np.where(kk < qq, 0.0, NEG).astype(np.float32)
    dmasks = {}
    for rho in (0, 1):
        m0 = m_inc
        m1 = m_inc if rho == 1 else m_exc
        dm = np.stack([np.stack([m0, m0], 0), np.stack([m1, m1], 0)], 0)
        # dm: [region, head, k, q] -> [k, region, head, q]
        dmasks[rho] = np.ascontiguousarray(dm.transpose(2, 0, 1, 3))

    in_maps = []
    for b in range(B):
        xf = x[b]  # [N, DIM]
        mv = mask_np[b].astype(np.float32)  # [N]
        for rho in (0, 1):
            perm_idx = np.concatenate(
                [np.arange(rho, N, 2), np.arange(1 - rho, N, 2)])
            xfT = np.ascontiguousarray(xf[perm_idx].T)
            mvp = mv[perm_idx]
            mvecT = np.ascontiguousarray(mvp.reshape(KC, 128).T)
            in_maps.append({
                "xfT": xfT, "wq": wq, "wk": wk, "wv": wv,
                "wout": w_out, "lng": lng, "dmask": dmasks[rho],
                "mvecT": mvecT,
            })
    return in_maps


_CACHE = {}
_LOCK = threading.Lock()


def _get_nc():
    with _LOCK:
        if "nc" not in _CACHE:
            _CACHE["nc"] = build()
    return _CACHE["nc"]


def kernel(x, mask, w_qkv, w_out, ln_g):
    in_maps = make_in_maps(x, mask, w_qkv, w_out, ln_g)
    nc = _get_nc()
    res = bass_utils.run_bass_kernel_spmd(nc, in_maps, core_ids=list(range(NC)))

    final = np.empty((B, N, DIM), dtype=np.float32)
    for b in range(B):
        for rho in (0, 1):
            final[b, rho::2, :] = res.results[2 * b + rho]["out"]
    return final


# revision 25
# speedup vs baseline: 1.4975x; 1.1764x over previous
"""CLIP attention block (LN(attn(x) @ W_out)) on 8 TRN2 NeuronCores.

Problem (hardcoded): x [4, 2048, 1024] f32, mask [4, 2048] bool,
w_qkv [1024, 3072], w_out [1024, 1024], ln_g [1024].
16 heads x 64 dim, causal, scale = 1/8. Output [4, 2048, 1024] f32.

Sharding: core = (batch b, parity rho); core computes output rows for
queries of batch b with token % 2 == rho (1024 tokens).

Layout: tokens are PERMUTED on host to [even | odd] so the core's query
set is a contiguous 1024-column slice of xfT and the causal structure
versus permuted key index k' is triangular with unit slope in each of
the two key regions (even keys: k' <= q'; odd keys: k' < q' for rho=0,
k' <= q' for rho=1). Keys/V/KT all live in permuted order (attention is
permutation-invariant over keys).

Pipeline (single pass, engineered for PE density / HAM warmth):
  prologue: V = x @ Wv (bf16, per-head-aug denominator column), K0/Q0.
  loop over 8 head pairs: packed 2-head score matmuls (f32r, K=64 row
  tiles at partitions 0/64 run concurrently), exp on ACT (scale=1/8
  folded in), AV matmuls in bf16 against V chunks (aug row 65
  accumulates the softmax denominator), per-head normalize via DVE
  reciprocal + gpsimd partition_broadcast; K/Q projection for the NEXT
  head pair is emitted between attention groups so the tensor engine
  always has dense independent work (keeps the HAM clock-gate at 8/8).
  Causally-dead column ranges are trimmed from score/AV streams and exp.
  epilogue: out proj over head pairs into 2x8 PSUM banks + layernorm.
"""

import threading

import numpy as np

import concourse.bass as bass
import concourse.mybir as mybir
import concourse.tile as tile
from concourse import bacc
import concourse.bass_utils as bass_utils

F32 = mybir.dt.float32
F32R = mybir.dt.float32r
BF16 = mybir.dt.bfloat16

B, N, DIM = 4, 2048, 1024
HEADS, DH = 16, 64
INNER = HEADS * DH          # 1024
SCALE = DH ** -0.5          # 0.125
LOC = N // 2                # 1024 local query tokens per core
NEG = -1.0e30
EPS = 1e-5

NC = 8                      # cores
HP = HEADS // 2             # 8 head pairs
KC = N // 128               # 16 key chunks (permuted order: 0-7 even, 8-15 odd)


def build(reps=1):
    nc = bacc.Bacc("TRN2", target_bir_lowering=False, debug=False, num_devices=NC)

    xfT = nc.dram_tensor("xfT", [DIM, N], F32R, kind="ExternalInput").ap()
    wq = nc.dram_tensor("wq", [DIM, INNER], F32R, kind="ExternalInput").ap()
    wk = nc.dram_tensor("wk", [DIM, INNER], F32R, kind="ExternalInput").ap()
    wv = nc.dram_tensor("wv", [DIM, INNER], F32R, kind="ExternalInput").ap()
    wout = nc.dram_tensor("wout", [INNER, DIM], F32R, kind="ExternalInput").ap()
    lng = nc.dram_tensor("lng", [1, DIM], F32, kind="ExternalInput").ap()
    dmask = nc.dram_tensor("dmask", [128, 2, 2, 128], F32, kind="ExternalInput").ap()
    mvecT = nc.dram_tensor("mvecT", [128, KC], F32, kind="ExternalInput").ap()
    out = nc.dram_tensor("out", [LOC, DIM], F32, kind="ExternalOutput").ap()

    with nc.allow_low_precision(reason="bf16 attention staging"), \
         tile.TileContext(nc) as tc:
        for _ in range(reps):
            _build_body(nc, tc, xfT, wq, wk, wv, wout, lng, dmask, mvecT, out)

    nc.compile()
    return nc


def _strips(gg):
    """Strips for q-group gg (512 local queries): (region, kci, trim s).

    region 0 = even keys (KT cols kci*128), region 1 = odd keys (KT cols
    (8+kci)*128). kci < 4*(gg+1). Diagonal strips (kci >= 4*gg) only have
    valid q columns [128*(kci-4*gg), 512)."""
    out = []
    for kci in range(4 * (gg + 1)):
        rel = kci - 4 * gg
        s = 128 * rel if rel >= 0 else 0
        out.append((0, kci, s))
        out.append((1, kci, s))
    return out


def _build_body(nc, tc, xfT, wq, wk, wv, wout, lng, dmask, mvecT, out):
    mm = nc.tensor.matmul
    A = mybir.ActivationFunctionType

    # ---------------- small residents ----------------
    res = tc.alloc_tile_pool(name="res", bufs=1)
    DMf = res.tile([128, 2, 2, 128], F32, tag="DMf")
    nc.sync.dma_start(DMf[:], dmask[:])
    DM = res.tile([128, 2, 2, 128], BF16, tag="DM")  # [k, region, head, q] 1/0
    nc.vector.tensor_copy(DM[:], DMf[:])
    MV = res.tile([128, KC], F32, tag="MV")
    nc.sync.dma_start(MV[:], mvecT[:])
    ones16 = res.tile([128, HEADS, 1], F32, tag="ones16")
    nc.vector.memset(ones16[:], 1.0)
    epst = res.tile([128, 1], F32, tag="epst")
    nc.vector.memset(epst[:], EPS)

    # V resident: 16 key chunks, per head [64 V cols | denominator col]
    vpool = tc.alloc_tile_pool(name="vres", bufs=1)
    V = [vpool.tile([128, HEADS, DH + 1], BF16, tag=f"V{i}", name=f"V{i}")
         for i in range(KC)]

    # OTS outlives the attention-phase pools below (LIFO release order)
    ots_pool = tc.alloc_tile_pool(name="ots", bufs=1)

    pps = tc.alloc_tile_pool(name="pps", bufs=2, space="PSUM")

    # ---------------- phase V: V projection ----------------
    xf_pool = tc.alloc_tile_pool(name="xf", bufs=1)
    XF = [xf_pool.tile([128, N], F32R, tag=f"xf{dc}", name=f"xf{dc}")
          for dc in range(8)]
    wv_pool = tc.alloc_tile_pool(name="wv", bufs=1)
    WV = [wv_pool.tile([128, INNER], F32R, tag=f"wv{dc}", name=f"wv{dc}")
          for dc in range(8)]
    qs = [nc.sync, nc.gpsimd, nc.scalar]
    for dc in range(8):  # all of XF first (everything waits on it)
        qs[dc % 3].dma_start(XF[dc][:], xfT[dc * 128:(dc + 1) * 128, :])
    for dc in range(8):
        qs[(dc + 1) % 2].dma_start(WV[dc][:], wv[dc * 128:(dc + 1) * 128, :])

    for tci in range(KC):
        for ig in range(2):
            vp = pps.tile([128, 512], F32, tag="pp", name="vp")
            for dc in range(8):
                mm(vp[:], XF[dc][:, tci * 128:(tci + 1) * 128],
                   WV[dc][:, ig * 512:(ig + 1) * 512],
                   start=(dc == 0), stop=(dc == 7))
            dst = V[tci][:, ig * 8:(ig + 1) * 8, 0:DH]
            nc.vector.tensor_scalar_mul(
                dst, vp[:].rearrange("p (h d) -> p h d", d=DH),
                MV[:, tci:tci + 1])
        nc.vector.tensor_scalar_mul(
            V[tci][:, :, DH:DH + 1], ones16[:], MV[:, tci:tci + 1])
    wv_pool.release()

    wq_r = wq.rearrange("(dc p) j -> p dc j", p=128)
    wk_r = wk.rearrange("(dc p) j -> p dc j", p=128)

    # ---------------- loop pools ----------------
    wkc_pool = tc.alloc_tile_pool(name="wkc", bufs=2)
    wqc_pool = tc.alloc_tile_pool(name="wqc", bufs=2)
    ktp_pool = tc.alloc_tile_pool(name="ktp", bufs=2)
    qtp_pool = tc.alloc_tile_pool(name="qtp", bufs=2)
    pt_pool = tc.alloc_tile_pool(name="pt", bufs=3)
    rc_pool = tc.alloc_tile_pool(name="rc", bufs=2)
    st_ps = tc.alloc_tile_pool(name="stps", bufs=2, space="PSUM")
    o_ps = tc.alloc_tile_pool(name="ops", bufs=1, space="PSUM")

    kts, qts = {}, {}

    def emit_kq(hp, part):
        """Emit K/Q projection work for head pair hp. part 0: wk dma +
        first 2 K chains; part 1: rest of K + Q."""
        if part == 0:
            wkc = wkc_pool.tile([128, 8, 128], F32R, tag="wkc", name="wkc")
            nc.gpsimd.dma_start(wkc[:], wk_r[:, :, hp * 128:(hp + 1) * 128])
            emit_kq.wkc = wkc
            wqc = wqc_pool.tile([128, 8, 128], F32R, tag="wqc", name="wqc")
            nc.sync.dma_start(wqc[:], wq_r[:, :, hp * 128:(hp + 1) * 128])
            emit_kq.wqc = wqc
            KTt = ktp_pool.tile([128, N], F32R, tag="ktt", name="KTt")
            kts[hp] = KTt
            tgs = range(0, 2)
        else:
            wkc = emit_kq.wkc
            KTt = kts[hp]
            tgs = range(2, 4)
        for tg in tgs:
            kp = pps.tile([128, 512], F32, tag="pp", name="kp")
            for dc in range(8):
                mm(kp[:], wkc[:, dc, :], XF[dc][:, tg * 512:(tg + 1) * 512],
                   start=(dc == 0), stop=(dc == 7))
            nc.vector.tensor_copy(KTt[:, tg * 512:(tg + 1) * 512], kp[:])
        if part == 1:
            wqc = emit_kq.wqc
            QTt = qtp_pool.tile([128, LOC], F32R, tag="qtt", name="QTt")
            qts[hp] = QTt
            for tg in range(2):
                qp = pps.tile([128, 512], F32, tag="pp", name="qp")
                for dc in range(8):
                    # queries = first half of the permuted tokens
                    mm(qp[:], wqc[:, dc, :],
                       XF[dc][:, tg * 512:(tg + 1) * 512],
                       start=(dc == 0), stop=(dc == 7))
                nc.vector.tensor_copy(QTt[:, tg * 512:(tg + 1) * 512], qp[:])

    emit_kq(0, 0)
    emit_kq(0, 1)

    # ---------------- attention loop ----------------
    OTS = []
    for hp in range(HP):
        KTt, QTt = kts.pop(hp), qts.pop(hp)
        OTt = ots_pool.tile([128, LOC], F32R, tag=f"ott{hp}", name=f"OTt{hp}")
        OTS.append(OTt)
        # denominator rows staged at partitions 0/32/64/96 (engine AP rule)
        D4 = rc_pool.tile([97, 512], F32, tag="d4", name="D4")
        nc.gpsimd.memset(D4[:], 1.0)
        for gg in range(2):
            strips = _strips(gg)
            nstrip = len(strips)
            ops = [o_ps.tile([DH + 1, 512], F32, tag=f"o{h}", name=f"op{h}")
                   for h in range(2)]
            sts = {}

            def emit_scores(t):
                region, kci, s = strips[t]
                ck = kci if region == 0 else 8 + kci
                st2 = st_ps.tile([128, 2, 512], F32, tag="st", name="st")
                sts[t] = st2
                qsl = slice(gg * 512 + s, (gg + 1) * 512)
                for h in range(2):
                    hs = slice(h * DH, (h + 1) * DH)
                    mm(st2[:, h, s:512],
                       KTt[hs, ck * 128:(ck + 1) * 128], QTt[hs, qsl],
                       start=True, stop=True)
                return st2

            for t in range(nstrip):
                if t == 0:
                    emit_scores(0)
                    if nstrip > 1:
                        emit_scores(1)
                elif t + 1 < nstrip:
                    emit_scores(t + 1)
                region, kci, s = strips[t]
                ck = kci if region == 0 else 8 + kci
                st2 = sts.pop(t)
                rel = kci - 4 * gg
                pt = pt_pool.tile([128, 2, 512], BF16, tag="pt", name="pt")
                nc.scalar.activation(pt[:, :, s:512], st2[:, :, s:512],
                                     A.Exp, scale=SCALE)
                if rel >= 0:  # diagonal strip: zero masked band post-exp
                    nc.vector.tensor_mul(
                        pt[:, :, s:s + 128], pt[:, :, s:s + 128],
                        DM[:, region])
                for h in range(2):
                    mm(ops[h][:, s:512], V[ck][:, 2 * hp + h, :],
                       pt[:, h, s:512],
                       start=(t == 0), stop=(t == nstrip - 1))
            # fast copies free the op PSUM banks; normalization is deferred
            for h in range(2):
                r = 32 * (2 * gg + h)
                nc.vector.tensor_copy(D4[r:r + 1, :], ops[h][DH:DH + 1, :])
                nc.vector.tensor_copy(
                    OTt[h * DH:(h + 1) * DH, gg * 512:(gg + 1) * 512],
                    ops[h][0:DH, :])
            # interleave next head pair's projections with attention
            if hp + 1 < HP:
                emit_kq(hp + 1, gg)
        # deferred per-head-pair normalization (off the PE critical path)
        R4 = rc_pool.tile([97, 512], F32, tag="r4", name="R4")
        nc.vector.reciprocal(R4[:], D4[:])
        for gg in range(2):
            for h in range(2):
                r = 32 * (2 * gg + h)
                stg = rc_pool.tile([1, 512], F32, tag="stg", name="stg")
                nc.vector.tensor_copy(stg[:], R4[r:r + 1, :])
                rbs = rc_pool.tile([128, 512], F32, tag="rbs", name="rbs")
                nc.gpsimd.partition_broadcast(rbs[:], stg[:])
                osl = OTt[h * DH:(h + 1) * DH, gg * 512:(gg + 1) * 512]
                nc.vector.tensor_mul(osl, osl, rbs[h * DH:(h + 1) * DH, :])

    o_ps.release()
    st_ps.release()
    rc_pool.release()
    pt_pool.release()
    qtp_pool.release()
    ktp_pool.release()
    wqc_pool.release()
    wkc_pool.release()
    xf_pool.release()
    pps.release()

    # ---------------- out projection + layernorm ----------------
    gz_pool = tc.alloc_tile_pool(name="gz", bufs=1)
    grow = gz_pool.tile([1, DIM], F32, tag="grow")
    nc.sync.dma_start(grow[:], lng[:])
    GB = gz_pool.tile([128, DIM], F32, tag="GB")
    nc.gpsimd.partition_broadcast(GB[:], grow[:])
    wo_pool = tc.alloc_tile_pool(name="wo", bufs=2)
    stat_pool = tc.alloc_tile_pool(name="stat", bufs=2)
    stage_pool = tc.alloc_tile_pool(name="stage", bufs=2)
    z_ps = tc.alloc_tile_pool(name="zps", bufs=1, space="PSUM")

    for tbg in range(2):
        zps = {}
        for ti in range(4):
            for half in range(2):
                zps[(ti, half)] = z_ps.tile([128, 512], F32, tag=f"z{ti}{half}",
                                            name=f"z{ti}{half}")
        for hp in range(HP):
            wo = wo_pool.tile([128, DIM], F32R, tag="wo", name="wo")
            nc.sync.dma_start(wo[:], wout[hp * 128:(hp + 1) * 128, :])
            for ti in range(4):
                tb = tbg * 4 + ti
                for half in range(2):
                    mm(zps[(ti, half)][:],
                       OTS[hp][:, tb * 128:(tb + 1) * 128],
                       wo[:, half * 512:(half + 1) * 512],
                       start=(hp == 0), stop=(hp == HP - 1))
        for ti in range(4):
            tb = tbg * 4 + ti
            s_ = [stat_pool.tile([128, 1], F32, tag=f"s{half}", name=f"s{half}")
                  for half in range(2)]
            q_ = [stat_pool.tile([128, 1], F32, tag=f"q{half}", name=f"q{half}")
                  for half in range(2)]
            scr = stage_pool.tile([128, 512], F32, tag="scr", name="scr")
            for half in range(2):
                nc.vector.reduce_sum(s_[half][:], zps[(ti, half)][:],
                                     axis=mybir.AxisListType.X)
                nc.scalar.activation(scr[:], zps[(ti, half)][:], A.Square,
                                     accum_out=q_[half][:])
            mean = stat_pool.tile([128, 1], F32, tag="mean", name="mean")
            nc.vector.tensor_add(mean[:], s_[0][:], s_[1][:])
            nc.vector.tensor_scalar_mul(mean[:], mean[:], 1.0 / DIM)
            msq = stat_pool.tile([128, 1], F32, tag="msq", name="msq")
            nc.vector.tensor_add(msq[:], q_[0][:], q_[1][:])
            nc.vector.tensor_scalar_mul(msq[:], msq[:], 1.0 / DIM)
            var = stat_pool.tile([128, 1], F32, tag="var", name="var")
            nc.vector.tensor_mul(var[:], mean[:], mean[:])
            nc.vector.tensor_sub(var[:], msq[:], var[:])
            std = stat_pool.tile([128, 1], F32, tag="std", name="std")
            nc.scalar.activation(std[:], var[:], A.Sqrt, bias=epst[:])
            rstd = stat_pool.tile([128, 1], F32, tag="rstd", name="rstd")
            nc.vector.reciprocal(rstd[:], std[:])
            nmr = stat_pool.tile([128, 1], F32, tag="nmr", name="nmr")
            nc.vector.tensor_mul(nmr[:], mean[:], rstd[:])
            nc.vector.tensor_scalar_mul(nmr[:], nmr[:], -1.0)
            outb = stage_pool.tile([128, DIM], F32, tag="outb", name="outb")
            for half in range(2):
                zn = stage_pool.tile([128, 512], F32, tag=f"zn{half}",
                                     name=f"zn{half}")
                nc.scalar.activation(zn[:], zps[(ti, half)][:], A.Identity,
                                     bias=nmr[:], scale=rstd[:])
                nc.vector.tensor_mul(outb[:, half * 512:(half + 1) * 512],
                                     zn[:], GB[:, half * 512:(half + 1) * 512])
            nc.gpsimd.dma_start(out[tb * 128:(tb + 1) * 128, :], outb[:])

    z_ps.release()
    stage_pool.release()
    stat_pool.release()
    wo_pool.release()
    gz_pool.release()
    ots_pool.release()
    vpool.release()
    res.release()


def make_in_maps(x, mask, w_qkv, w_out, ln_g):
    x = np.asarray(x, dtype=np.float32)
    mask_np = np.asarray(mask)
    w_qkv = np.asarray(w_qkv, dtype=np.float32)
    w_out = np.ascontiguousarray(np.asarray(w_out, dtype=np.float32))
    ln_g = np.asarray(ln_g, dtype=np.float32)

    wq = np.ascontiguousarray(w_qkv[:, :INNER])
    wk = np.ascontiguousarray(w_qkv[:, INNER:2 * INNER])
    wv = np.ascontiguousarray(w_qkv[:, 2 * INNER:])
    lng = np.ascontiguousarray(ln_g[None, :])

    # Diagonal masks in permuted space. Core's queries are parity rho and
    # are placed FIRST in the permutation [rho tokens | 1-rho tokens], so
    # the query slice is always columns 0..1023. Region 0 = same-parity
    # keys (global 2k'+rho vs 2q'+rho: k' <= q'), region 1 = other-parity
    # keys (global 2k'+(1-rho) vs 2q'+rho: valid iff 2k'+1-rho <= 2q'+rho,
    # i.e. k' < q' + rho: rho=0 -> k' < q'; rho=1 -> k' <= q').
    kk = np.arange(128)[:, None]
    qq = np.arange(128)[None, :]
    m_inc = (kk <= qq).astype(np.float32)   # multiplicative post-exp masks
    m_exc = (kk < qq).astype(np.float32)
    dmasks = {}
    for rho in (0, 1):
        m0 = m_inc
        m1 = m_inc if rho == 1 else m_exc
        dm = np.stack([np.stack([m0, m0], 0), np.stack([m1, m1], 0)], 0)
        # dm: [region, head, k, q] -> [k, region, head, q]
        dmasks[rho] = np.ascontiguousarray(
            dm.transpose(2, 0, 1, 3).astype(np.float32))

    in_maps = []
    for b in range(B):
        xf = x[b]  # [N, DIM]
        mv = mask_np[b].astype(np.float32)  # [N]
        for rho in (0, 1):
            perm_idx = np.concatenate(
                [np.arange(rho, N, 2), np.arange(1 - rho, N, 2)])
            xfT = np.ascontiguousarray(xf[perm_idx].T)
            mvp = mv[perm_idx]
            mvecT = np.ascontiguousarray(mvp.reshape(KC, 128).T)
            in_maps.append({
                "xfT": xfT, "wq": wq, "wk": wk, "wv": wv,
                "wout": w_out, "lng": lng, "dmask": dmasks[rho],
                "mvecT": mvecT,
            })
    return in_maps


_CACHE = {}
_LOCK = threading.Lock()


def _get_nc():
    with _LOCK:
        if "nc" not in _CACHE:
            _CACHE["nc"] = build()
    return _CACHE["nc"]


def kernel(x, mask, w_qkv, w_out, ln_g):
    in_maps = make_in_maps(x, mask, w_qkv, w_out, ln_g)
    nc = _get_nc()
    res = bass_utils.run_bass_kernel_spmd(nc, in_maps, core_ids=list(range(NC)))

    final = np.empty((B, N, DIM), dtype=np.float32)
    for b in range(B):
        for rho in (0, 1):
            final[b, rho::2, :] = res.results[2 * b + rho]["out"]
    return final
